# revision 1
# baseline (speedup 1.0000x reference)
"""MDM denoiser (RoPE transformer) Trainium2 kernel.

Sharding: data-parallel over batch; each NeuronCore runs the full 8-layer
transformer on 4 sequences (2048 tokens). No collectives.

v2 vs baseline:
- residual h kept in bf16 (no fp32->bf16 cast passes for LN stats)
- LN rstd = Exp(-0.5*Ln(var+eps)); softmax 1/denom = Exp(-Ln(d)) -- all
  scalar-engine transcendentals stay in the natural_log_exp table set,
  so only gelu forces a table switch (2 loads/layer vs ~17).
- rotate-half produced by 4 SBUF->SBUF DMA partition-block swaps with the
  sign pattern folded into the sin table: the 2*D rot weight columns and
  their matmuls are gone.
- chunk-group-major GEMM loops: one stationary load streams 2 chunk
  columns (and 4 for QKV/FF via paired PSUM bufs).
- one PSUM pool of 4x [128,2,512] pair tiles; exp/stats/gelu run as
  paired ops spanning 2 banks.
"""

import os
import sys

for _p in (
    "/root/.axon_site",
    "/root/.axon_site/_ro/trn_rl_repo",
    "/root/.axon_site/_ro/pypackages",
    "/opt/trn_rl_repo",
):
    if os.path.isdir(_p) and _p not in sys.path:
        sys.path.append(_p)

import ml_dtypes
import numpy as np

import concourse.bass as bass
import concourse.tile as tile
from concourse import mybir
from concourse.bass import ds, ts
from concourse.bass_utils import run_bass_kernel_spmd
from concourse.vector_clock import ScopedClock

BF16 = ml_dtypes.bfloat16
F32 = mybir.dt.float32
BF = mybir.dt.bfloat16

B, T, D_IN = 32, 512, 150
D, L, H = 512, 8, 8
HD = D // H          # 64
FF = 4 * D           # 2048
LLM, TXT = 512, 20
NCORES = 8
BL = B // NCORES     # 4 sequences per core
TOK = BL * T         # 2048 tokens per core
P = 128
KD = D // P          # 4
KF = FF // P         # 16
EPS = 1e-5

Alu = mybir.AluOpType
Act = mybir.ActivationFunctionType


class _TileContext(tile.TileContext):
    """TileContext whose kernel-tail drain is compatible with this walrus
    (one sync wait per NO_STRUCT instruction)."""

    def _drain_and_barrier(self, tick_clock, wait_clock):
        probe = self.nc.sync.nop()
        wait_clock.add_sem_waits(
            probe.ins, ScopedClock({None: tick_clock.global_clock})
        )
        si = probe.ins.sync_info
        waits = list(si.on_wait) if si is not None else []
        probe.ins.sync_info = mybir.SyncInfo(on_wait=waits[:1], on_update=[])
        for w in waits[1:]:
            n = self.nc.sync.nop()
            n.ins.sync_info = mybir.SyncInfo(on_wait=[w], on_update=[])
        self.nc.sync.drain()
        self.nc.all_engine_barrier()
        assert self.sems is not None
        popped = self.nc._tile_sem_poison_stack.pop()
        assert popped is self._sem_poison
        self.nc.clear_and_free_semaphores(list(self.sems.allocated().values()))
        self.nc.all_engine_barrier()


def _split_sync_waits(nc):
    """Encode at most one sync wait per instruction; hoist extras onto
    preceding same-engine NOPs."""
    nid = 0
    for fn in nc.m.functions:
        for bb in fn.blocks:
            out = []
            for ins in bb.instructions:
                si = getattr(ins, "sync_info", None)
                if si is not None and len(si.on_wait) > 1:
                    waits = list(si.on_wait)
                    for w in waits[:-1]:
                        nop = mybir.InstNoOp(name=f"I-sw{nid}", ins=[], outs=[])
                        nid += 1
                        nop.engine = ins.engine
                        nop.sync_info = mybir.SyncInfo(on_wait=[w], on_update=[])
                        out.append(nop)
                    ins.sync_info = mybir.SyncInfo(
                        on_wait=[waits[-1]], on_update=list(si.on_update)
                    )
                out.append(ins)
            bb.instructions = out


# ---------------------------------------------------------------------------
# device program
# ---------------------------------------------------------------------------

def _build_nc():
    nc = bass.Bass(target_bir_lowering=False)

    # ---- DRAM tensors -----------------------------------------------------
    x_fm = nc.dram_tensor("x_fm", [P, 2, TOK], BF, kind="ExternalInput")
    enc_fm = nc.dram_tensor("enc_fm", [P, KD, BL, TXT], F32, kind="ExternalInput")
    onehot = nc.dram_tensor("onehot", [P, 8, BL], BF, kind="ExternalInput")
    pe_tab = nc.dram_tensor("pe_tab", [P, 8, D], BF, kind="ExternalInput")
    w_t1 = nc.dram_tensor("w_t1", [P, KD, D], BF, kind="ExternalInput")
    w_t2 = nc.dram_tensor("w_t2", [P, KD, D], BF, kind="ExternalInput")
    w_txt = nc.dram_tensor("w_txt", [P, KD, D], BF, kind="ExternalInput")
    w_in = nc.dram_tensor("w_in", [P, 2, D], BF, kind="ExternalInput")
    w_qk = nc.dram_tensor("w_qk", [L, P, KD, 2 * D], BF, kind="ExternalInput")
    w_v = nc.dram_tensor("w_v", [L, P, KD, D], BF, kind="ExternalInput")
    w_o = nc.dram_tensor("w_o", [L, P, KD, D], BF, kind="ExternalInput")
    w_1 = nc.dram_tensor("w_1", [L, P, KD, FF], BF, kind="ExternalInput")
    w_2 = nc.dram_tensor("w_2", [L, P, KF, D], BF, kind="ExternalInput")
    w_out = nc.dram_tensor("w_out", [P, KD, D_IN], BF, kind="ExternalInput")
    cos_t = nc.dram_tensor("cos_t", [P, 2, T], BF, kind="ExternalInput")
    sinm_t = nc.dram_tensor("sinm_t", [P, 2, T], BF, kind="ExternalInput")
    # biases: cols = bqk(0:8) bo(8:12) b1(12:28) b2(28:32)
    blk = nc.dram_tensor("blk", [L, P, 32], F32, kind="ExternalInput")
    b_v = nc.dram_tensor("b_v", [L, 1, H * HD], BF, kind="ExternalInput")
    bt1_fm = nc.dram_tensor("bt1_fm", [P, 4], F32, kind="ExternalInput")
    bemb_fm = nc.dram_tensor("bemb_fm", [P, 4], F32, kind="ExternalInput")
    bout_fm = nc.dram_tensor("bout_fm", [P, 2], F32, kind="ExternalInput")
    out_d = nc.dram_tensor("out", [D_IN, TOK], F32, kind="ExternalOutput")

    from contextlib import ExitStack

    with _TileContext(nc) as tc, ExitStack() as ctx:
        ep = ctx.enter_context
        singles = ep(tc.tile_pool(name="singles", bufs=1))
        hp = ep(tc.tile_pool(name="hp", bufs=1))
        psW = ep(tc.tile_pool(name="psW", bufs=2, space="PSUM"))

        def pstile(name):
            return psW.tile([P, 2, T], F32, tag="ps", name=name)

        def psftile(name):
            return psW.tile([P, 2, T], F32, tag="psF", name=name)

        # ---- constants ----
        cos_sb = singles.tile([P, 2, T], BF)
        nc.sync.dma_start(cos_sb[:], cos_t[:])
        sinm_sb = singles.tile([P, 2, T], BF)
        nc.sync.dma_start(sinm_sb[:], sinm_t[:])
        ones_bf = singles.tile([P, 1], BF)
        nc.vector.memset(ones_bf[:], 1.0)
        ones1 = singles.tile([1, P], BF)
        nc.vector.memset(ones1[:], 1.0)
        eps_sb = singles.tile([1, 1], F32)
        nc.vector.memset(eps_sb[:], EPS)
        bout_sb = singles.tile([P, 2], F32)
        nc.sync.dma_start(bout_sb[:], bout_fm[:])
        bt1_sb = singles.tile([P, 4], F32)
        nc.sync.dma_start(bt1_sb[:], bt1_fm[:])
        bemb_sb = singles.tile([P, 4], F32)
        nc.sync.dma_start(bemb_sb[:], bemb_fm[:])

        # ---- conditioning: timestep PE -> MLP, text mean -> linear ----
        condp = tc.tile_pool(name="condp", bufs=1)
        cp = condp.__enter__()
        pe_sb = cp.tile([P, 8, D], BF, name="pe_sb")
        nc.sync.dma_start(pe_sb[:], pe_tab[:])
        wt1_sb = cp.tile([P, KD, D], BF, name="wt1_sb")
        nc.sync.dma_start(wt1_sb[:], w_t1[:])
        wt2_sb = cp.tile([P, KD, D], BF, name="wt2_sb")
        nc.sync.dma_start(wt2_sb[:], w_t2[:])
        wtxt_sb = cp.tile([P, KD, D], BF, name="wtxt_sb")
        nc.sync.dma_start(wtxt_sb[:], w_txt[:])
        oh_sb = cp.tile([P, 8, BL], BF, name="oh_sb")
        nc.sync.dma_start(oh_sb[:], onehot[:])
        enc_sb = cp.tile([P, KD, BL, TXT], F32, name="enc_sb")
        nc.sync.dma_start(enc_sb[:], enc_fm[:])

        # gather timestep PE rows via one-hot matmul
        tpe_sb = cp.tile([P, KD, BL], BF, name="tpe_sb")
        for dt in range(KD):
            acc = pstile("c_tpe")
            for o in range(8):
                nc.tensor.matmul(
                    acc[:, 0, 0:BL], pe_sb[:, o, ts(dt, P)], oh_sb[:, o, :],
                    start=(o == 0), stop=(o == 7),
                )
            nc.vector.tensor_copy(tpe_sb[:, dt, :], acc[:, 0, 0:BL])

        # t1 = silu(pe @ W_t1 + b_t1)
        t1_sb = cp.tile([P, KD, BL], BF, name="t1_sb")
        for dt in range(KD):
            acc = pstile("c_t1")
            for k in range(KD):
                nc.tensor.matmul(
                    acc[:, 0, 0:BL], wt1_sb[:, k, ts(dt, P)], tpe_sb[:, k, :],
                    start=(k == 0), stop=(k == KD - 1),
                )
            nc.scalar.activation(
                t1_sb[:, dt, :], acc[:, 0, 0:BL], Act.Silu,
                bias=bt1_sb[:, dt : dt + 1],
            )

        # text mean (sum; /TXT folded into W_txt on host)
        encr = cp.tile([P, KD, BL], F32, name="encr")
        for k in range(KD):
            nc.vector.reduce_sum(
                encr[:, k, :], enc_sb[:, k, :, :], axis=mybir.AxisListType.X
            )
        encb = cp.tile([P, KD, BL], BF, name="encb")
        nc.vector.tensor_copy(encb[:], encr[:])

        # emb = t1 @ W_t2 + txtsum @ (W_txt/TXT) + (b_t2 + b_txt + b_in)
        emb_sb = singles.tile([P, KD, BL], F32)
        for dt in range(KD):
            acc = pstile("c_emb")
            for k in range(KD):
                nc.tensor.matmul(
                    acc[:, 0, 0:BL], wt2_sb[:, k, ts(dt, P)], t1_sb[:, k, :],
                    start=(k == 0), stop=False,
                )
            for k in range(KD):
                nc.tensor.matmul(
                    acc[:, 0, 0:BL], wtxt_sb[:, k, ts(dt, P)], encb[:, k, :],
                    start=False, stop=(k == KD - 1),
                )
            nc.vector.tensor_scalar(
                emb_sb[:, dt, :], acc[:, 0, 0:BL], bemb_sb[:, dt : dt + 1],
                None, Alu.add,
            )

        # ---- input projection: h = x @ W_in + emb (b_in inside emb) ----
        x_sb = cp.tile([P, 2, TOK], BF, name="x_sb")
        nc.sync.dma_start(x_sb[:], x_fm[:])
        win_sb = cp.tile([P, 2, D], BF, name="win_sb")
        nc.sync.dma_start(win_sb[:], w_in[:])
        h = hp.tile([P, KD, TOK], BF)
        for g in range(2):
            for dt in range(KD):
                acc = pstile("inp")
                for k in range(2):
                    for c2 in range(2):
                        nc.tensor.matmul(
                            acc[:, c2, :], win_sb[:, k, ts(dt, P)],
                            x_sb[:, k, ds((2 * g + c2) * T, T)],
                            start=(k == 0), stop=(k == 1),
                        )
                for c2 in range(2):
                    c = 2 * g + c2
                    nc.vector.tensor_scalar(
                        h[:, dt, ds(c * T, T)], acc[:, c2, :],
                        emb_sb[:, dt, c : c + 1], None, Alu.add,
                    )

        condp.__exit__(None, None, None)
        wts = ep(tc.tile_pool(name="wts", bufs=2))
        wff = ep(tc.tile_pool(name="wff", bufs=1))
        lnp = ep(tc.tile_pool(name="lnp", bufs=1))
        qkp = ep(tc.tile_pool(name="qkp", bufs=1))
        rp = ep(tc.tile_pool(name="rp", bufs=2))
        ap = ep(tc.tile_pool(name="ap", bufs=2))
        st2 = ep(tc.tile_pool(name="st2", bufs=1))
        outp = ep(tc.tile_pool(name="outp", bufs=1))

        # ---------------------------------------------------------------
        def ln_stats(g, tag):
            """Sums + scalar chain for chunk group g -> (negm, rstd) tiles."""
            gsl = ds(g * 2 * T, 2 * T)
            hsq = lnp.tile([P, KD, 2 * T], BF, tag="hsq", name="hsq",
                           bufs=2)
            nc.vector.tensor_tensor(hsq[:], h[:, :, gsl], h[:, :, gsl],
                                    Alu.mult)
            sts = pstile(f"st_s{tag}")
            stq = pstile(f"st_q{tag}")
            for c2 in range(2):
                cs = ds((2 * g + c2) * T, T)
                for k in range(KD):
                    nc.tensor.matmul(
                        sts[0:1, c2, :], ones_bf[:], h[:, k, cs],
                        start=(k == 0), stop=(k == KD - 1),
                    )
                for k in range(KD):
                    nc.tensor.matmul(
                        stq[0:1, c2, :], ones_bf[:], hsq[:, k, ds(c2 * T, T)],
                        start=(k == 0), stop=(k == KD - 1),
                    )
            negm = st2.tile([1, 2, T], BF, tag="negm", name="negm", bufs=2)
            nc.scalar.activation(negm[:], sts[0:1], Act.Copy, scale=-1.0 / D)
            m2 = st2.tile([1, 2, T], F32, tag="m2", name="m2", bufs=1)
            nc.scalar.activation(m2[:], sts[0:1], Act.Square, scale=1.0 / D)
            nc.vector.scalar_tensor_tensor(
                m2[:], stq[0:1], 1.0 / D, m2[:], Alu.mult, Alu.subtract,
            )
            nc.scalar.activation(m2[:], m2[:], Act.Ln, bias=eps_sb[:])
            rstd = st2.tile([1, 2, T], BF, tag="rstd", name="rstd", bufs=2)
            nc.scalar.activation(rstd[:], m2[:], Act.Exp, scale=-0.5)
            return negm, rstd

        def ln_apply(g, stats, y_t, tag):
            """y[group g] = (h - mean) * rstd (gains folded into weights)."""
            negm, rstd = stats
            gsl = ds(g * 2 * T, 2 * T)
            nb = psftile(f"nb{tag}")
            for c2 in range(2):
                nc.tensor.matmul(nb[:, c2, :], ones1[:], negm[:, c2, :],
                                 start=True, stop=True)
            for k in range(KD):
                nc.vector.tensor_tensor(y_t[:, k, gsl], h[:, k, gsl], nb[:],
                                        Alu.add)
            rb = psftile(f"rb{tag}")
            for c2 in range(2):
                nc.tensor.matmul(rb[:, c2, :], ones1[:], rstd[:, c2, :],
                                 start=True, stop=True)
            rb_sb = st2.tile([P, 2, T], BF, tag="rbsb", name="rbsb", bufs=2)
            nc.scalar.activation(rb_sb[:], rb[:], Act.Copy)
            for k in range(KD):
                nc.vector.tensor_tensor(y_t[:, k, gsl], y_t[:, k, gsl],
                                        rb_sb[:], Alu.mult)

        y = lnp.tile([P, KD, TOK], BF, tag="y", name="y")

        # ---- layer loop, software-pipelined LN ----
        # Each LN's stats are emitted early (scalar chain overlaps dense PE
        # phases); its apply lands just before the consumer GEMM.
        ln_apply(0, ln_stats(0, "boot"), y, "boot")

        for l in range(L):
            wqk_sb = wts.tile([P, KD, 2 * D], BF, tag="wqk", name="wqk",
                              bufs=1)
            nc.sync.dma_start(wqk_sb[:], w_qk[l])
            wv_sb = wts.tile([P, KD, D], BF, tag="wv", name="wv", bufs=1)
            nc.sync.dma_start(wv_sb[:], w_v[l])
            wo_sb = wts.tile([P, KD, D], BF, tag="wo", name="wo", bufs=1)
            nc.sync.dma_start(wo_sb[:], w_o[l])
            blk_sb = wts.tile([P, 32], F32, tag="blk", name="blk")
            nc.sync.dma_start(blk_sb[:], blk[l])
            bv_row = wts.tile([1, H * HD], BF, tag="bvr", name="bvr")
            nc.sync.dma_start(bv_row[:], b_v[l])
            w1_sb = wff.tile([P, KD, FF], BF, tag="w1", name="w1")
            nc.sync.dma_start(w1_sb[:], w_1[l])
            w2_sb = wff.tile([P, KF, D], BF, tag="w2", name="w2")
            nc.sync.dma_start(w2_sb[:], w_2[l])
            bqk_c = blk_sb[:, 0:8]
            bo_c = blk_sb[:, 8:12]
            b1_c = blk_sb[:, 12:28]
            b2_c = blk_sb[:, 28:32]

            # bv broadcast tile (token-major V bias, per layer)
            bvb_ps = psftile("bvb")
            for c2 in range(2):
                nc.tensor.matmul(bvb_ps[:, c2, :], ones1[:], bv_row[:],
                                 start=True, stop=True)
            bv_b = wts.tile([P, 2, T], BF, tag="bvb", name="bvb", bufs=1)
            nc.scalar.activation(bv_b[:], bvb_ps[:], Act.Copy)

            def qkv_rope(g):
                for dt in range(2 * KD):
                    qkv_rope_dt(g, dt)

            def qkv_rope_dt(g, dt):
                if True:
                    acc = pstile(f"qk{dt % 2}")
                    for k in range(KD):
                        for c2 in range(2):
                            nc.tensor.matmul(
                                acc[:, c2, :],
                                wqk_sb[:, k, ds(dt * P, P)],
                                y[:, k, ds((2 * g + c2) * T, T)],
                                start=(k == 0), stop=(k == KD - 1),
                            )
                    pre = rp.tile([P, 2, T], BF, tag="pre", name="pre")
                    if dt % 2 == 0:
                        nc.vector.tensor_scalar(
                            pre[:], acc[:], bqk_c[:, dt : dt + 1], None,
                            Alu.add,
                        )
                    else:
                        nc.scalar.activation(
                            pre[:], acc[:], Act.Identity,
                            bias=bqk_c[:, dt : dt + 1],
                        )
                    rot = rp.tile([P, 2, T], BF, tag="rot", name="rot")
                    nc.sync.dma_start(rot[0:32], pre[32:64])
                    nc.sync.dma_start(rot[32:64], pre[0:32])
                    nc.sync.dma_start(rot[64:96], pre[96:128])
                    nc.sync.dma_start(rot[96:128], pre[64:96])
                    t1 = rp.tile([P, 2, T], BF, tag="t1", name="t1", bufs=1)
                    nc.vector.tensor_tensor(t1[:], pre[:], cos_sb[:], Alu.mult)
                    t2 = rp.tile([P, 2, T], BF, tag="t2", name="t2", bufs=1)
                    nc.vector.tensor_tensor(t2[:], rot[:], sinm_sb[:],
                                            Alu.mult)
                    dst = qk_q if dt < KD else qk_k
                    nc.vector.tensor_tensor(
                        dst[:, dt % KD, :], t1[:], t2[:], Alu.add
                    )

            def vgemm(c, va):
                nc.vector.memset(va[:, :, :, HD : HD + 1], 1.0)
                for qq in range(2):
                    vgemm_qq(c, va, qq)

            def vgemm_qq(c, va, qq):
                if True:
                    acc = pstile(f"v{qq}")
                    for q2 in range(2):
                        q = 2 * qq + q2
                        for k in range(KD):
                            nc.tensor.matmul(
                                acc[:, q2, :],
                                y[:, k, ds(c * T + q * P, P)],
                                wv_sb[:, k, :],
                                start=(k == 0), stop=(k == KD - 1),
                            )
                    nc.vector.tensor_tensor(
                        va[:, 2 * qq : 2 * qq + 2, :, 0:HD],
                        acc[:].rearrange("p a (h d) -> p a h d", h=H),
                        bv_b[:].rearrange("p a (h d) -> p a h d", h=H),
                        Alu.add,
                    )

            def attn_pair(hp_, c2, va, osb):
                """QK+exp for head pair hp_ of chunk c2; returns p tiles."""
                pA = ap.tile([P, KD, T], BF, tag="pp", name="pA", bufs=4)
                pB = ap.tile([P, KD, T], BF, tag="pp", name="pB", bufs=4)
                qa = qk_q[0:HD, hp_, ds(c2 * T, T)]
                ka = qk_k[0:HD, hp_, :]
                qb = qk_q[HD:P, hp_, ds(c2 * T, T)]
                kb = qk_k[HD:P, hp_, :]
                for half in range(2):
                    sca = pstile("sca")
                    scb = pstile("scb")
                    for kt2 in range(2):
                        kt = 2 * half + kt2
                        ksl = ds(c2 * T + kt * P, P)
                        nc.tensor.matmul(sca[:, kt2, :], ka[:, ksl],
                                         qa, start=True, stop=True)
                        nc.tensor.matmul(scb[:, kt2, :], kb[:, ksl],
                                         qb, start=True, stop=True)
                    nc.scalar.activation(
                        pA[:, 2 * half : 2 * half + 2, :], sca[:],
                        Act.Exp, scale=0.125,
                    )
                    nc.scalar.activation(
                        pB[:, 2 * half : 2 * half + 2, :], scb[:],
                        Act.Exp, scale=0.125,
                    )
                return pA, pB

            def attn_tail(hp_, c2, va, osb, pA, pB):
                po = psftile("po")
                for kt in range(KD):
                    nc.tensor.matmul(
                        po[0 : HD + 1, 0, :], va[:, kt, 2 * hp_, :],
                        pA[:, kt, :],
                        start=(kt == 0), stop=(kt == KD - 1),
                    )
                for kt in range(KD):
                    nc.tensor.matmul(
                        po[0 : HD + 1, 1, :], va[:, kt, 2 * hp_ + 1, :],
                        pB[:, kt, :],
                        start=(kt == 0), stop=(kt == KD - 1),
                    )
                lnd = st2.tile([1, 2, T], BF, tag="lnd", name="lnd", bufs=2)
                nc.scalar.activation(lnd[:], po[HD : HD + 1, :, :], Act.Ln)
                rbf = st2.tile([1, 2, T], BF, tag="rbf", name="rbf", bufs=2)
                nc.scalar.activation(rbf[:], lnd[:], Act.Exp, scale=-1.0)
                rbb = psftile("rbb")
                for j in range(2):
                    nc.tensor.matmul(rbb[:, j, :], ones1[:],
                                     rbf[:, j, :], start=True, stop=True)
                oraw = st2.tile([HD, 2, T], BF, tag="oraw", name="oraw",
                                bufs=2)
                nc.vector.tensor_copy(oraw[:], po[0:HD, :, :])
                nc.vector.tensor_tensor(
                    osb[0:HD, hp_, :], oraw[:, 0, :], rbb[0:HD, 0, :],
                    Alu.mult,
                )
                nc.vector.tensor_tensor(
                    osb[HD:P, hp_, :], oraw[:, 1, :], rbb[0:HD, 1, :],
                    Alu.mult,
                )

            def attn_group(va0, va1, o0, o1):
                for hp_ in range(4):
                    p0 = attn_pair(hp_, 0, va0, o0)
                    p1 = attn_pair(hp_, 1, va1, o1)
                    attn_tail(hp_, 0, va0, o0, *p0)
                    attn_tail(hp_, 1, va1, o1, *p1)

            def oproj(g, o0, o1):
                gsl = ds(g * 2 * T, 2 * T)
                for dt in range(KD):
                    acc = pstile("opj")
                    for k in range(KD):
                        nc.tensor.matmul(
                            acc[:, 0, :], wo_sb[:, k, ts(dt, P)],
                            o0[:, k, :],
                            start=(k == 0), stop=(k == KD - 1),
                        )
                    for k in range(KD):
                        nc.tensor.matmul(
                            acc[:, 1, :], wo_sb[:, k, ts(dt, P)],
                            o1[:, k, :],
                            start=(k == 0), stop=(k == KD - 1),
                        )
                    nc.vector.scalar_tensor_tensor(
                        h[:, dt, gsl].rearrange("p (a b) -> p a b", a=2),
                        acc[:], bo_c[:, dt : dt + 1],
                        h[:, dt, gsl].rearrange("p (a b) -> p a b", a=2),
                        Alu.add, Alu.add,
                    )

            def ff(g):
                for c2 in range(2):
                    c = 2 * g + c2
                    cs = ds(c * T, T)
                    f2a = psftile("f2a")
                    f2b = psftile("f2b")
                    f2 = [f2a[:, 0, :], f2a[:, 1, :], f2b[:, 0, :],
                          f2b[:, 1, :]]

                    for k in range(KF):
                        acc = pstile(f"f1{k % 2}")
                        for kk in range(KD):
                            nc.tensor.matmul(
                                acc[:, k % 2, :], w1_sb[:, kk, ts(k, P)],
                                y[:, kk, cs],
                                start=(kk == 0), stop=(kk == KD - 1),
                            )
                        gk = rp.tile([P, T], BF, tag="gk", name="gk")
                        nc.scalar.activation(
                            gk[:], acc[:, k % 2, :], Act.Gelu,
                            bias=b1_c[:, k : k + 1],
                        )
                        for dt in range(KD):
                            nc.tensor.matmul(
                                f2[dt], w2_sb[:, k, ts(dt, P)], gk[:],
                                start=(k == 0), stop=(k == KF - 1),
                            )
                    for dt in range(KD):
                        nc.vector.scalar_tensor_tensor(
                            h[:, dt, cs], f2[dt], b2_c[:, dt : dt + 1],
                            h[:, dt, cs], Alu.add, Alu.add,
                        )

            # ---- pipelined schedule ----
            def layernorm_group(g, tag):
                ln_apply(g, ln_stats(g, tag), y, tag)

            layernorm_group(1, "u1g1")     # LN1(g1) overlaps QKV(g0)+attn
            qk_q = qkp.tile([P, KD, 2 * T], BF, tag="qq", name="qk_q")
            qk_k = qkp.tile([P, KD, 2 * T], BF, tag="qk", name="qk_k")
            qkv_rope(0)
            va0 = ap.tile([P, KD, H, HD + 1], BF, tag="va", name="va0")
            va1 = ap.tile([P, KD, H, HD + 1], BF, tag="va", name="va1")
            vgemm(0, va0)
            vgemm(1, va1)
            o00 = ap.tile([P, KD, T], BF, tag="o0", name="o00", bufs=1)
            o01 = ap.tile([P, KD, T], BF, tag="o1", name="o01", bufs=1)
            attn_group(va0, va1, o00, o01)
            oproj(0, o00, o01)
            layernorm_group(0, "u2g0")     # LN2(g0) overlaps QKV(g1)
            qkv_rope(1)
            va0 = ap.tile([P, KD, H, HD + 1], BF, tag="va", name="va0b")
            va1 = ap.tile([P, KD, H, HD + 1], BF, tag="va", name="va1b")
            vgemm(2, va0)
            vgemm(3, va1)
            o00 = ap.tile([P, KD, T], BF, tag="o0", name="o10", bufs=1)
            o01 = ap.tile([P, KD, T], BF, tag="o1", name="o11", bufs=1)
            attn_group(va0, va1, o00, o01)
            oproj(1, o00, o01)
            layernorm_group(1, "u2g1")     # LN2(g1) overlaps FF(g0)
            ff(0)
            if l < L - 1:
                layernorm_group(0, "nl")   # next layer LN1(g0) overlaps FF(g1)
            ff(1)

        # ---- output projection ----
        wout_sb = singles.tile([P, KD, D_IN], BF)
        nc.sync.dma_start(wout_sb[:], w_out[:])
        for c in range(BL):
            cs = ds(c * T, T)
            acc1 = pstile("o1")
            for k in range(KD):
                nc.tensor.matmul(
                    acc1[:, 0, :], wout_sb[:, k, 0:P], h[:, k, cs],
                    start=(k == 0), stop=(k == KD - 1),
                )
            for k in range(KD):
                nc.tensor.matmul(
                    acc1[0 : D_IN - P, 1, :], wout_sb[:, k, P:D_IN],
                    h[:, k, cs],
                    start=(k == 0), stop=(k == KD - 1),
                )
            o1 = outp.tile([P, T], F32, tag="o1t", name="o1")
            nc.vector.tensor_scalar(
                o1[:], acc1[:, 0, :], bout_sb[:, 0:1], None, Alu.add
            )
            nc.sync.dma_start(out_d[0:P, cs], o1[:])
            o2 = outp.tile([P, T], F32, tag="o1t", name="o2")
            nc.vector.tensor_scalar(
                o2[0 : D_IN - P, :], acc1[0 : D_IN - P, 1, :],
                bout_sb[0 : D_IN - P, 1:2], None, Alu.add,
            )
            nc.sync.dma_start(out_d[P:D_IN, cs], o2[0 : D_IN - P, :])

    _split_sync_waits(nc)
    return nc


# ---------------------------------------------------------------------------
# host-side preparation
# ---------------------------------------------------------------------------

def _fm(w):
    """[K, N] -> [128, K//128, N] (partition-major k-tiles)."""
    k, n = w.shape
    return np.ascontiguousarray(w.reshape(k // P, P, n).transpose(1, 0, 2))


def _bias_fm(v):
    """[n*128] -> [128, n] feature-major per-partition columns."""
    return np.ascontiguousarray(v.reshape(-1, P).T)


def _prep_shared(inputs):
    f32 = np.float32
    g = {}

    # rope tables; sinm has the rotate-half sign pattern folded in
    inv = 1.0 / (10000.0 ** (np.arange(0, HD, 2, dtype=f32) / HD))
    ang = np.arange(T, dtype=f32)[:, None] * inv[None, :]
    ang = np.concatenate([ang, ang], axis=-1)          # [T, HD]
    cos = np.cos(ang).T.astype(f32)                    # [HD, T]
    sin = np.sin(ang).T.astype(f32)
    sgn = np.where(np.arange(HD) < HD // 2, -1.0, 1.0).astype(f32)
    sinm = sin * sgn[:, None]
    cos128 = np.concatenate([cos, cos], axis=0)        # [128, T]
    sinm128 = np.concatenate([sinm, sinm], axis=0)
    g["cos_t"] = np.repeat(cos128[:, None, :], 2, axis=1).astype(BF16)
    g["sinm_t"] = np.repeat(sinm128[:, None, :], 2, axis=1).astype(BF16)

    # timestep sinusoidal PE table for t in 0..1023
    pos = np.arange(1024, dtype=f32)[:, None]
    div = np.exp(-np.log(10000.0) * np.arange(0, D, 2, dtype=f32) / D)
    a = pos * div[None, :]
    tab = np.stack([np.sin(a), np.cos(a)], axis=-1).reshape(1024, D).astype(f32)
    g["pe_tab"] = np.ascontiguousarray(
        tab.reshape(8, P, D).transpose(1, 0, 2)
    ).astype(BF16)

    W_t1 = np.asarray(inputs["W_t1"], f32)
    W_t2 = np.asarray(inputs["W_t2"], f32)
    W_txt = np.asarray(inputs["W_txt"], f32)
    g["w_t1"] = _fm(W_t1).astype(BF16)
    g["w_t2"] = _fm(W_t2).astype(BF16)
    g["w_txt"] = _fm(W_txt / TXT).astype(BF16)
    g["bt1_fm"] = _bias_fm(np.asarray(inputs["b_t1"], f32))
    bemb = (
        np.asarray(inputs["b_t2"], f32)
        + np.asarray(inputs["b_txt"], f32)
        + np.asarray(inputs["b_in"], f32)
    )
    g["bemb_fm"] = _bias_fm(bemb)

    W_in = np.asarray(inputs["W_in"], f32)
    w_in_pad = np.zeros((2 * P, D), f32)
    w_in_pad[:D_IN] = W_in
    g["w_in"] = _fm(w_in_pad).astype(BF16)

    W_out = np.asarray(inputs["W_out"], f32)
    g["w_out"] = _fm(W_out).astype(BF16)
    b_out = np.asarray(inputs["b_out"], f32)
    bo_fm = np.zeros((P, 2), f32)
    bo_fm[:, 0] = b_out[:P]
    bo_fm[: D_IN - P, 1] = b_out[P:]
    g["bout_fm"] = bo_fm

    Wqkv = np.asarray(inputs["Wqkv"], f32)
    bqkv = np.asarray(inputs["bqkv"], f32)
    ln1_g = np.asarray(inputs["ln1_g"], f32)
    ln1_b = np.asarray(inputs["ln1_b"], f32)
    ln2_g = np.asarray(inputs["ln2_g"], f32)
    ln2_b = np.asarray(inputs["ln2_b"], f32)
    Wo = np.asarray(inputs["Wo"], f32)
    bo = np.asarray(inputs["bo"], f32)
    W1 = np.asarray(inputs["W1"], f32)
    b1 = np.asarray(inputs["b1"], f32)
    W2 = np.asarray(inputs["W2"], f32)
    b2 = np.asarray(inputs["b2"], f32)

    w_qk_l, w_v_l, w_o_l, w_1_l, w_2_l, blk_l, bv_l = [], [], [], [], [], [], []
    for l in range(L):
        Wq = Wqkv[l][:, 0:D]
        Wk = Wqkv[l][:, D : 2 * D]
        Wv = Wqkv[l][:, 2 * D : 3 * D]
        # LN gains folded into the weight rows (y on device is plain xhat)
        Wq_e = ln1_g[l][:, None] * Wq
        Wk_e = ln1_g[l][:, None] * Wk
        Wv_e = ln1_g[l][:, None] * Wv
        W1_e = ln2_g[l][:, None] * W1[l]
        bq_eff = bqkv[l][0:D] + ln1_b[l] @ Wq
        bk_eff = bqkv[l][D : 2 * D] + ln1_b[l] @ Wk
        bv_eff = bqkv[l][2 * D : 3 * D] + ln1_b[l] @ Wv
        b1_eff = b1[l] + ln2_b[l] @ W1[l]
        w_qk_l.append(_fm(np.concatenate([Wq_e, Wk_e], axis=1)).astype(BF16))
        w_v_l.append(_fm(Wv_e).astype(BF16))
        w_o_l.append(_fm(Wo[l]).astype(BF16))
        w_1_l.append(_fm(W1_e).astype(BF16))
        w_2_l.append(_fm(W2[l]).astype(BF16))
        blk_one = np.concatenate(
            [
                _bias_fm(bq_eff), _bias_fm(bk_eff),
                _bias_fm(bo[l]), _bias_fm(b1_eff), _bias_fm(b2[l]),
            ],
            axis=1,
        )  # [128, 32]
        blk_l.append(blk_one)
        bv_l.append(bv_eff)
    g["w_qk"] = np.stack(w_qk_l)
    g["w_v"] = np.stack(w_v_l)
    g["w_o"] = np.stack(w_o_l)
    g["w_1"] = np.stack(w_1_l)
    g["w_2"] = np.stack(w_2_l)
    g["blk"] = np.stack(blk_l).astype(np.float32)
    g["b_v"] = np.stack(bv_l).astype(BF16)[:, None, :]
    return g


def _prep_core(inputs, cc):
    f32 = np.float32
    d = {}
    bs = slice(cc * BL, (cc + 1) * BL)

    x = np.asarray(inputs["x"], f32)[bs]
    x_t = x.reshape(TOK, D_IN).T
    x_pad = np.zeros((2 * P, TOK), f32)
    x_pad[:D_IN] = x_t
    d["x_fm"] = x_pad.reshape(2, P, TOK).transpose(1, 0, 2).astype(BF16)

    enc = np.asarray(inputs["enc_text"], f32)[bs]
    enc_fm = enc.transpose(2, 0, 1)
    d["enc_fm"] = np.ascontiguousarray(
        enc_fm.reshape(KD, P, BL, TXT).transpose(1, 0, 2, 3)
    )

    tsv = np.asarray(inputs["timesteps"]).astype(np.int64)[bs]
    oh = np.zeros((P, 8, BL), f32)
    for j, t in enumerate(tsv):
        oh[int(t) % P, int(t) // P, j] = 1.0
    d["onehot"] = oh.astype(BF16)
    return d


_CACHE = {}


def kernel(**inputs):
    if "nc" not in _CACHE:
        _CACHE["nc"] = _build_nc()
    nc = _CACHE["nc"]

    shared = _prep_shared(inputs)
    in_maps = []
    for cc in range(NCORES):
        m = dict(shared)
        m.update(_prep_core(inputs, cc))
        in_maps.append(m)

    res = run_bass_kernel_spmd(
        nc, in_maps, core_ids=list(range(NCORES)), **_CACHE.get("run_kwargs", {})
    )
    _CACHE["last_result"] = res

    outs = []
    for cc in range(NCORES):
        o = res.results[cc]["out"]
        outs.append(o.reshape(D_IN, BL, T).transpose(1, 2, 0))
    return np.ascontiguousarray(np.concatenate(outs, axis=0), dtype=np.float32)



# revision 3
# speedup vs baseline: 1.1739x; 1.1739x over previous
"""MDM denoiser (RoPE transformer) Trainium2 kernel.

Sharding: data-parallel over batch; each NeuronCore runs the full 8-layer
transformer on 4 sequences (2048 tokens). No collectives.

v2 vs baseline:
- residual h kept in bf16 (no fp32->bf16 cast passes for LN stats)
- LN rstd = Exp(-0.5*Ln(var+eps)); softmax 1/denom = Exp(-Ln(d)) -- all
  scalar-engine transcendentals stay in the natural_log_exp table set,
  so only gelu forces a table switch (2 loads/layer vs ~17).
- rotate-half produced by 4 SBUF->SBUF DMA partition-block swaps with the
  sign pattern folded into the sin table: the 2*D rot weight columns and
  their matmuls are gone.
- chunk-group-major GEMM loops: one stationary load streams 2 chunk
  columns (and 4 for QKV/FF via paired PSUM bufs).
- one PSUM pool of 4x [128,2,512] pair tiles; exp/stats/gelu run as
  paired ops spanning 2 banks.
"""

import os
import sys

for _p in (
    "/root/.axon_site",
    "/root/.axon_site/_ro/trn_rl_repo",
    "/root/.axon_site/_ro/pypackages",
    "/opt/trn_rl_repo",
):
    if os.path.isdir(_p) and _p not in sys.path:
        sys.path.append(_p)

import ml_dtypes
import numpy as np

import concourse.bass as bass
import concourse.tile as tile
from concourse import mybir
from concourse.bass import ds, ts
from concourse.bass_utils import run_bass_kernel_spmd
from concourse.vector_clock import ScopedClock

BF16 = ml_dtypes.bfloat16
F32 = mybir.dt.float32
BF = mybir.dt.bfloat16

B, T, D_IN = 32, 512, 150
D, L, H = 512, 8, 8
HD = D // H          # 64
FF = 4 * D           # 2048
LLM, TXT = 512, 20
NCORES = 8
BL = B // NCORES     # 4 sequences per core
TOK = BL * T         # 2048 tokens per core
P = 128
KD = D // P          # 4
KF = FF // P         # 16
EPS = 1e-5

Alu = mybir.AluOpType
Act = mybir.ActivationFunctionType


class _TileContext(tile.TileContext):
    """TileContext whose kernel-tail drain is compatible with this walrus
    (one sync wait per NO_STRUCT instruction)."""

    def _drain_and_barrier(self, tick_clock, wait_clock):
        probe = self.nc.sync.nop()
        wait_clock.add_sem_waits(
            probe.ins, ScopedClock({None: tick_clock.global_clock})
        )
        si = probe.ins.sync_info
        waits = list(si.on_wait) if si is not None else []
        probe.ins.sync_info = mybir.SyncInfo(on_wait=waits[:1], on_update=[])
        for w in waits[1:]:
            n = self.nc.sync.nop()
            n.ins.sync_info = mybir.SyncInfo(on_wait=[w], on_update=[])
        self.nc.sync.drain()
        self.nc.all_engine_barrier()
        assert self.sems is not None
        popped = self.nc._tile_sem_poison_stack.pop()
        assert popped is self._sem_poison
        self.nc.clear_and_free_semaphores(list(self.sems.allocated().values()))
        self.nc.all_engine_barrier()


def _split_sync_waits(nc):
    """Encode at most one sync wait per instruction; hoist extras onto
    preceding same-engine NOPs."""
    nid = 0
    for fn in nc.m.functions:
        for bb in fn.blocks:
            out = []
            for ins in bb.instructions:
                si = getattr(ins, "sync_info", None)
                if si is not None and len(si.on_wait) > 1:
                    waits = list(si.on_wait)
                    for w in waits[:-1]:
                        nop = mybir.InstNoOp(name=f"I-sw{nid}", ins=[], outs=[])
                        nid += 1
                        nop.engine = ins.engine
                        nop.sync_info = mybir.SyncInfo(on_wait=[w], on_update=[])
                        out.append(nop)
                    ins.sync_info = mybir.SyncInfo(
                        on_wait=[waits[-1]], on_update=list(si.on_update)
                    )
                out.append(ins)
            bb.instructions = out


# ---------------------------------------------------------------------------
# device program
# ---------------------------------------------------------------------------

_PHASES = []


def _build_nc():
    nc = bass.Bass(target_bir_lowering=False)

    def mk(tag):
        _PHASES.append((tag, nc.next_id()))

    # ---- DRAM tensors -----------------------------------------------------
    x_fm = nc.dram_tensor("x_fm", [P, 2, TOK], BF, kind="ExternalInput")
    enc_fm = nc.dram_tensor("enc_fm", [P, KD, BL, TXT], F32, kind="ExternalInput")
    onehot = nc.dram_tensor("onehot", [P, 8, BL], BF, kind="ExternalInput")
    pe_tab = nc.dram_tensor("pe_tab", [P, 8, D], BF, kind="ExternalInput")
    w_t1 = nc.dram_tensor("w_t1", [P, KD, D], BF, kind="ExternalInput")
    w_t2 = nc.dram_tensor("w_t2", [P, KD, D], BF, kind="ExternalInput")
    w_txt = nc.dram_tensor("w_txt", [P, KD, D], BF, kind="ExternalInput")
    w_in = nc.dram_tensor("w_in", [P, 2, D], BF, kind="ExternalInput")
    w_qk = nc.dram_tensor("w_qk", [L, P, KD, 2 * D], BF, kind="ExternalInput")
    w_v = nc.dram_tensor("w_v", [L, P, KD, D], BF, kind="ExternalInput")
    w_o = nc.dram_tensor("w_o", [L, P, KD, D], BF, kind="ExternalInput")
    w_1 = nc.dram_tensor("w_1", [L, P, KD, FF], BF, kind="ExternalInput")
    w_2 = nc.dram_tensor("w_2", [L, P, KF, D], BF, kind="ExternalInput")
    w_out = nc.dram_tensor("w_out", [P, KD, D_IN], BF, kind="ExternalInput")
    cos_t = nc.dram_tensor("cos_t", [P, 2, T], BF, kind="ExternalInput")
    sinm_t = nc.dram_tensor("sinm_t", [P, 2, T], BF, kind="ExternalInput")
    # biases: cols = bqk(0:8) bo(8:12) b1(12:28) b2(28:32)
    blk = nc.dram_tensor("blk", [L, P, 32], F32, kind="ExternalInput")
    b_v = nc.dram_tensor("b_v", [L, 1, H * HD], BF, kind="ExternalInput")
    bt1_fm = nc.dram_tensor("bt1_fm", [P, 4], F32, kind="ExternalInput")
    bemb_fm = nc.dram_tensor("bemb_fm", [P, 4], F32, kind="ExternalInput")
    bout_fm = nc.dram_tensor("bout_fm", [P, 2], F32, kind="ExternalInput")
    out_d = nc.dram_tensor("out", [D_IN, TOK], F32, kind="ExternalOutput")

    from contextlib import ExitStack

    with _TileContext(nc) as tc, ExitStack() as ctx:
        ep = ctx.enter_context
        singles = ep(tc.tile_pool(name="singles", bufs=1))
        hp = ep(tc.tile_pool(name="hp", bufs=1))
        psW = ep(tc.tile_pool(name="psW", bufs=2, space="PSUM"))

        def pstile(name):
            return psW.tile([P, 2, T], F32, tag="ps", name=name)

        def psftile(name):
            return psW.tile([P, 2, T], F32, tag="psF", name=name)

        # ---- constants ----
        cos_sb = singles.tile([P, 2, T], BF)
        nc.sync.dma_start(cos_sb[:], cos_t[:])
        sinm_sb = singles.tile([P, 2, T], BF)
        nc.sync.dma_start(sinm_sb[:], sinm_t[:])
        ones_bf = singles.tile([P, 1], BF)
        nc.vector.memset(ones_bf[:], 1.0)
        ones1 = singles.tile([1, P], BF)
        nc.vector.memset(ones1[:], 1.0)
        eps_sb = singles.tile([1, 1], F32)
        nc.vector.memset(eps_sb[:], EPS)
        bout_sb = singles.tile([P, 2], F32)
        nc.sync.dma_start(bout_sb[:], bout_fm[:])
        bt1_sb = singles.tile([P, 4], F32)
        nc.sync.dma_start(bt1_sb[:], bt1_fm[:])
        bemb_sb = singles.tile([P, 4], F32)
        nc.sync.dma_start(bemb_sb[:], bemb_fm[:])

        mk("cond")
        # ---- conditioning: timestep PE -> MLP, text mean -> linear ----
        condp = tc.tile_pool(name="condp", bufs=1)
        cp = condp.__enter__()
        pe_sb = cp.tile([P, 8, D], BF, name="pe_sb")
        nc.sync.dma_start(pe_sb[:], pe_tab[:])
        wt1_sb = cp.tile([P, KD, D], BF, name="wt1_sb")
        nc.sync.dma_start(wt1_sb[:], w_t1[:])
        wt2_sb = cp.tile([P, KD, D], BF, name="wt2_sb")
        nc.sync.dma_start(wt2_sb[:], w_t2[:])
        wtxt_sb = cp.tile([P, KD, D], BF, name="wtxt_sb")
        nc.sync.dma_start(wtxt_sb[:], w_txt[:])
        oh_sb = cp.tile([P, 8, BL], BF, name="oh_sb")
        nc.sync.dma_start(oh_sb[:], onehot[:])
        enc_sb = cp.tile([P, KD, BL, TXT], F32, name="enc_sb")
        nc.sync.dma_start(enc_sb[:], enc_fm[:])

        # gather timestep PE rows via one-hot matmul
        tpe_sb = cp.tile([P, KD, BL], BF, name="tpe_sb")
        for dt in range(KD):
            acc = pstile("c_tpe")
            for o in range(8):
                nc.tensor.matmul(
                    acc[:, 0, 0:BL], pe_sb[:, o, ts(dt, P)], oh_sb[:, o, :],
                    start=(o == 0), stop=(o == 7),
                )
            nc.vector.tensor_copy(tpe_sb[:, dt, :], acc[:, 0, 0:BL])

        # t1 = silu(pe @ W_t1 + b_t1)
        t1_sb = cp.tile([P, KD, BL], BF, name="t1_sb")
        for dt in range(KD):
            acc = pstile("c_t1")
            for k in range(KD):
                nc.tensor.matmul(
                    acc[:, 0, 0:BL], wt1_sb[:, k, ts(dt, P)], tpe_sb[:, k, :],
                    start=(k == 0), stop=(k == KD - 1),
                )
            nc.scalar.activation(
                t1_sb[:, dt, :], acc[:, 0, 0:BL], Act.Silu,
                bias=bt1_sb[:, dt : dt + 1],
            )

        # text mean (sum; /TXT folded into W_txt on host)
        encr = cp.tile([P, KD, BL], F32, name="encr")
        for k in range(KD):
            nc.vector.reduce_sum(
                encr[:, k, :], enc_sb[:, k, :, :], axis=mybir.AxisListType.X
            )
        encb = cp.tile([P, KD, BL], BF, name="encb")
        nc.vector.tensor_copy(encb[:], encr[:])

        # emb = t1 @ W_t2 + txtsum @ (W_txt/TXT) + (b_t2 + b_txt + b_in)
        emb_sb = singles.tile([P, KD, BL], F32)
        for dt in range(KD):
            acc = pstile("c_emb")
            for k in range(KD):
                nc.tensor.matmul(
                    acc[:, 0, 0:BL], wt2_sb[:, k, ts(dt, P)], t1_sb[:, k, :],
                    start=(k == 0), stop=False,
                )
            for k in range(KD):
                nc.tensor.matmul(
                    acc[:, 0, 0:BL], wtxt_sb[:, k, ts(dt, P)], encb[:, k, :],
                    start=False, stop=(k == KD - 1),
                )
            nc.vector.tensor_scalar(
                emb_sb[:, dt, :], acc[:, 0, 0:BL], bemb_sb[:, dt : dt + 1],
                None, Alu.add,
            )

        mk("inproj")
        # ---- input projection: h = x @ W_in + emb (b_in inside emb) ----
        x_sb = cp.tile([P, 2, TOK], BF, name="x_sb")
        nc.sync.dma_start(x_sb[:], x_fm[:])
        win_sb = cp.tile([P, 2, D], BF, name="win_sb")
        nc.sync.dma_start(win_sb[:], w_in[:])
        h = hp.tile([P, KD, TOK], BF)
        for g in range(2):
            for dt in range(KD):
                acc = pstile("inp")
                for k in range(2):
                    for c2 in range(2):
                        nc.tensor.matmul(
                            acc[:, c2, :], win_sb[:, k, ts(dt, P)],
                            x_sb[:, k, ds((2 * g + c2) * T, T)],
                            start=(k == 0), stop=(k == 1),
                        )
                for c2 in range(2):
                    c = 2 * g + c2
                    nc.vector.tensor_scalar(
                        h[:, dt, ds(c * T, T)], acc[:, c2, :],
                        emb_sb[:, dt, c : c + 1], None, Alu.add,
                    )

        condp.__exit__(None, None, None)
        wts = ep(tc.tile_pool(name="wts", bufs=2))
        wff = ep(tc.tile_pool(name="wff", bufs=1))
        lnp = ep(tc.tile_pool(name="lnp", bufs=1))
        qkp = ep(tc.tile_pool(name="qkp", bufs=1))
        rp = ep(tc.tile_pool(name="rp", bufs=2))
        ap = ep(tc.tile_pool(name="ap", bufs=2))
        st2 = ep(tc.tile_pool(name="st2", bufs=1))
        outp = ep(tc.tile_pool(name="outp", bufs=1))

        # ---------------------------------------------------------------
        def ln_stats(g, tag):
            """Sums + scalar chain for chunk group g -> (negm, rstd) tiles."""
            gsl = ds(g * 2 * T, 2 * T)
            hsq = lnp.tile([P, KD, 2 * T], BF, tag="hsq", name="hsq",
                           bufs=2)
            nc.vector.tensor_tensor(hsq[:], h[:, :, gsl], h[:, :, gsl],
                                    Alu.mult)
            sts = pstile(f"st_s{tag}")
            stq = pstile(f"st_q{tag}")
            for c2 in range(2):
                cs = ds((2 * g + c2) * T, T)
                for k in range(KD):
                    nc.tensor.matmul(
                        sts[0:1, c2, :], ones_bf[:], h[:, k, cs],
                        start=(k == 0), stop=(k == KD - 1),
                    )
                for k in range(KD):
                    nc.tensor.matmul(
                        stq[0:1, c2, :], ones_bf[:], hsq[:, k, ds(c2 * T, T)],
                        start=(k == 0), stop=(k == KD - 1),
                    )
            negm = st2.tile([1, 2, T], BF, tag="negm", name="negm", bufs=2)
            nc.scalar.activation(negm[:], sts[0:1], Act.Copy, scale=-1.0 / D)
            m2 = st2.tile([1, 2, T], F32, tag="m2", name="m2", bufs=1)
            nc.scalar.activation(m2[:], sts[0:1], Act.Square, scale=1.0 / D)
            nc.vector.scalar_tensor_tensor(
                m2[:], stq[0:1], 1.0 / D, m2[:], Alu.mult, Alu.subtract,
            )
            nc.scalar.activation(m2[:], m2[:], Act.Ln, bias=eps_sb[:])
            rstd = st2.tile([1, 2, T], BF, tag="rstd", name="rstd", bufs=2)
            nc.scalar.activation(rstd[:], m2[:], Act.Exp, scale=-0.5)
            return negm, rstd

        def ln_apply(g, stats, y_t, tag):
            """y[group g] = (h - mean) * rstd (gains folded into weights)."""
            negm, rstd = stats
            gsl = ds(g * 2 * T, 2 * T)
            nb = psftile(f"nb{tag}")
            for c2 in range(2):
                nc.tensor.matmul(nb[:, c2, :], ones1[:], negm[:, c2, :],
                                 start=True, stop=True)
            for k in range(KD):
                nc.vector.tensor_tensor(y_t[:, k, gsl], h[:, k, gsl], nb[:],
                                        Alu.add)
            rb = psftile(f"rb{tag}")
            for c2 in range(2):
                nc.tensor.matmul(rb[:, c2, :], ones1[:], rstd[:, c2, :],
                                 start=True, stop=True)
            rb_sb = st2.tile([P, 2, T], BF, tag="rbsb", name="rbsb", bufs=2)
            nc.scalar.activation(rb_sb[:], rb[:], Act.Copy)
            for k in range(KD):
                nc.vector.tensor_tensor(y_t[:, k, gsl], y_t[:, k, gsl],
                                        rb_sb[:], Alu.mult)

        y = lnp.tile([P, KD, TOK], BF, tag="y", name="y")

        # ---- layer loop, software-pipelined LN ----
        # Each LN's stats are emitted early (scalar chain overlaps dense PE
        # phases); its apply lands just before the consumer GEMM.
        mk("lnboot")
        ln_apply(0, ln_stats(0, "boot"), y, "boot")

        for l in range(L):
            mk(f"L{l}.wdma")
            wqk_sb = wts.tile([P, KD, 2 * D], BF, tag="wqk", name="wqk",
                              bufs=1)
            nc.sync.dma_start(wqk_sb[:], w_qk[l])
            wv_sb = wts.tile([P, KD, D], BF, tag="wv", name="wv", bufs=1)
            nc.sync.dma_start(wv_sb[:], w_v[l])
            wo_sb = wts.tile([P, KD, D], BF, tag="wo", name="wo", bufs=1)
            nc.sync.dma_start(wo_sb[:], w_o[l])
            blk_sb = wts.tile([P, 32], F32, tag="blk", name="blk")
            nc.sync.dma_start(blk_sb[:], blk[l])
            bv_row = wts.tile([1, H * HD], BF, tag="bvr", name="bvr")
            nc.sync.dma_start(bv_row[:], b_v[l])
            w1_sb = wff.tile([P, KD, FF], BF, tag="w1", name="w1")
            nc.sync.dma_start(w1_sb[:], w_1[l])
            w2_sb = wff.tile([P, KF, D], BF, tag="w2", name="w2")
            nc.sync.dma_start(w2_sb[:], w_2[l])
            bqk_c = blk_sb[:, 0:8]
            bo_c = blk_sb[:, 8:12]
            b1_c = blk_sb[:, 12:28]
            b2_c = blk_sb[:, 28:32]

            # bv broadcast tile (token-major V bias, per layer)
            bvb_ps = psftile("bvb")
            for c2 in range(2):
                nc.tensor.matmul(bvb_ps[:, c2, :], ones1[:], bv_row[:],
                                 start=True, stop=True)
            bv_b = wts.tile([P, 2, T], BF, tag="bvb", name="bvb", bufs=1)
            nc.scalar.activation(bv_b[:], bvb_ps[:], Act.Copy)

            def qkv_rope(g):
                for dt in range(2 * KD):
                    qkv_rope_dt(g, dt)

            def qkv_rope_dt(g, dt):
                if True:
                    acc = pstile(f"qk{dt % 2}")
                    for k in range(KD):
                        for c2 in range(2):
                            nc.tensor.matmul(
                                acc[:, c2, :],
                                wqk_sb[:, k, ds(dt * P, P)],
                                y[:, k, ds((2 * g + c2) * T, T)],
                                start=(k == 0), stop=(k == KD - 1),
                            )
                    pre = rp.tile([P, 2, T], BF, tag="pre", name="pre")
                    if dt % 2 == 0:
                        nc.vector.tensor_scalar(
                            pre[:], acc[:], bqk_c[:, dt : dt + 1], None,
                            Alu.add,
                        )
                    else:
                        nc.scalar.activation(
                            pre[:], acc[:], Act.Identity,
                            bias=bqk_c[:, dt : dt + 1],
                        )
                    rot = rp.tile([P, 2, T], BF, tag="rot", name="rot")
                    nc.sync.dma_start(rot[0:32], pre[32:64])
                    nc.sync.dma_start(rot[32:64], pre[0:32])
                    nc.sync.dma_start(rot[64:96], pre[96:128])
                    nc.sync.dma_start(rot[96:128], pre[64:96])
                    t1 = rp.tile([P, 2, T], BF, tag="t1", name="t1", bufs=1)
                    nc.vector.tensor_tensor(t1[:], pre[:], cos_sb[:], Alu.mult)
                    t2 = rp.tile([P, 2, T], BF, tag="t2", name="t2", bufs=1)
                    nc.vector.tensor_tensor(t2[:], rot[:], sinm_sb[:],
                                            Alu.mult)
                    dst = qk_q if dt < KD else qk_k
                    nc.vector.tensor_tensor(
                        dst[:, dt % KD, :], t1[:], t2[:], Alu.add
                    )

            def vgemm(c, va):
                nc.vector.memset(va[:, :, :, HD : HD + 1], 1.0)
                for qq in range(2):
                    vgemm_qq(c, va, qq)

            def vgemm_qq(c, va, qq):
                if True:
                    acc = pstile(f"v{qq}")
                    for q2 in range(2):
                        q = 2 * qq + q2
                        for k in range(KD):
                            nc.tensor.matmul(
                                acc[:, q2, :],
                                y[:, k, ds(c * T + q * P, P)],
                                wv_sb[:, k, :],
                                start=(k == 0), stop=(k == KD - 1),
                            )
                    nc.vector.tensor_tensor(
                        va[:, 2 * qq : 2 * qq + 2, :, 0:HD],
                        acc[:].rearrange("p a (h d) -> p a h d", h=H),
                        bv_b[:].rearrange("p a (h d) -> p a h d", h=H),
                        Alu.add,
                    )

            def attn_pair(hp_, c2, va, osb):
                """QK+exp for head pair hp_ of chunk c2; returns p tiles."""
                pA = ap.tile([P, KD, T], BF, tag="pp", name="pA", bufs=4)
                pB = ap.tile([P, KD, T], BF, tag="pp", name="pB", bufs=4)
                qa = qk_q[0:HD, hp_, ds(c2 * T, T)]
                ka = qk_k[0:HD, hp_, :]
                qb = qk_q[HD:P, hp_, ds(c2 * T, T)]
                kb = qk_k[HD:P, hp_, :]
                for half in range(2):
                    sca = pstile("sca")
                    scb = pstile("scb")
                    for kt2 in range(2):
                        kt = 2 * half + kt2
                        ksl = ds(c2 * T + kt * P, P)
                        nc.tensor.matmul(sca[:, kt2, :], ka[:, ksl],
                                         qa, start=True, stop=True)
                        nc.tensor.matmul(scb[:, kt2, :], kb[:, ksl],
                                         qb, start=True, stop=True)
                    nc.scalar.activation(
                        pA[:, 2 * half : 2 * half + 2, :], sca[:],
                        Act.Exp, scale=0.125,
                    )
                    nc.scalar.activation(
                        pB[:, 2 * half : 2 * half + 2, :], scb[:],
                        Act.Exp, scale=0.125,
                    )
                return pA, pB

            def attn_tail(hp_, c2, va, osb, pA, pB):
                po = psftile("po")
                for kt in range(KD):
                    nc.tensor.matmul(
                        po[0 : HD + 1, 0, :], va[:, kt, 2 * hp_, :],
                        pA[:, kt, :],
                        start=(kt == 0), stop=(kt == KD - 1),
                    )
                for kt in range(KD):
                    nc.tensor.matmul(
                        po[0 : HD + 1, 1, :], va[:, kt, 2 * hp_ + 1, :],
                        pB[:, kt, :],
                        start=(kt == 0), stop=(kt == KD - 1),
                    )
                lnd = st2.tile([1, 2, T], BF, tag="lnd", name="lnd", bufs=2)
                nc.scalar.activation(lnd[:], po[HD : HD + 1, :, :], Act.Ln)
                rbf = st2.tile([1, 2, T], BF, tag="rbf", name="rbf", bufs=2)
                nc.scalar.activation(rbf[:], lnd[:], Act.Exp, scale=-1.0)
                rbb = psftile("rbb")
                for j in range(2):
                    nc.tensor.matmul(rbb[:, j, :], ones1[:],
                                     rbf[:, j, :], start=True, stop=True)
                oraw = st2.tile([HD, 2, T], BF, tag="oraw", name="oraw",
                                bufs=2)
                nc.vector.tensor_copy(oraw[:], po[0:HD, :, :])
                nc.vector.tensor_tensor(
                    osb[0:HD, hp_, :], oraw[:, 0, :], rbb[0:HD, 0, :],
                    Alu.mult,
                )
                nc.vector.tensor_tensor(
                    osb[HD:P, hp_, :], oraw[:, 1, :], rbb[0:HD, 1, :],
                    Alu.mult,
                )

            def attn_group(va0, va1, o0, o1):
                for hp_ in range(4):
                    p0 = attn_pair(hp_, 0, va0, o0)
                    p1 = attn_pair(hp_, 1, va1, o1)
                    attn_tail(hp_, 0, va0, o0, *p0)
                    attn_tail(hp_, 1, va1, o1, *p1)

            def oproj(g, o0, o1):
                gsl = ds(g * 2 * T, 2 * T)
                for dt in range(KD):
                    acc = pstile("opj")
                    for k in range(KD):
                        nc.tensor.matmul(
                            acc[:, 0, :], wo_sb[:, k, ts(dt, P)],
                            o0[:, k, :],
                            start=(k == 0), stop=(k == KD - 1),
                        )
                    for k in range(KD):
                        nc.tensor.matmul(
                            acc[:, 1, :], wo_sb[:, k, ts(dt, P)],
                            o1[:, k, :],
                            start=(k == 0), stop=(k == KD - 1),
                        )
                    nc.vector.scalar_tensor_tensor(
                        h[:, dt, gsl].rearrange("p (a b) -> p a b", a=2),
                        acc[:], bo_c[:, dt : dt + 1],
                        h[:, dt, gsl].rearrange("p (a b) -> p a b", a=2),
                        Alu.add, Alu.add,
                    )

            def ff(g):
                for c2 in range(2):
                    c = 2 * g + c2
                    cs = ds(c * T, T)
                    f2a = psftile("f2a")
                    f2b = psftile("f2b")
                    f2 = [f2a[:, 0, :], f2a[:, 1, :], f2b[:, 0, :],
                          f2b[:, 1, :]]

                    for k in range(KF):
                        acc = pstile(f"f1{k % 2}")
                        for kk in range(KD):
                            nc.tensor.matmul(
                                acc[:, k % 2, :], w1_sb[:, kk, ts(k, P)],
                                y[:, kk, cs],
                                start=(kk == 0), stop=(kk == KD - 1),
                            )
                        gk = rp.tile([P, T], BF, tag="gk", name="gk")
                        nc.scalar.activation(
                            gk[:], acc[:, k % 2, :], Act.Gelu,
                            bias=b1_c[:, k : k + 1],
                        )
                        for dt in range(KD):
                            nc.tensor.matmul(
                                f2[dt], w2_sb[:, k, ts(dt, P)], gk[:],
                                start=(k == 0), stop=(k == KF - 1),
                            )
                    for dt in range(KD):
                        nc.vector.scalar_tensor_tensor(
                            h[:, dt, cs], f2[dt], b2_c[:, dt : dt + 1],
                            h[:, dt, cs], Alu.add, Alu.add,
                        )

            # ---- pipelined schedule ----
            def layernorm_group(g, tag):
                ln_apply(g, ln_stats(g, tag), y, tag)

            mk(f"L{l}.lnA")
            layernorm_group(1, "u1g1")     # LN1(g1) overlaps QKV(g0)+attn
            mk(f"L{l}.qkv0")
            qk_q = qkp.tile([P, KD, 2 * T], BF, tag="qq", name="qk_q")
            qk_k = qkp.tile([P, KD, 2 * T], BF, tag="qk", name="qk_k")
            qkv_rope(0)
            mk(f"L{l}.vg0")
            va0 = ap.tile([P, KD, H, HD + 1], BF, tag="va", name="va0")
            va1 = ap.tile([P, KD, H, HD + 1], BF, tag="va", name="va1")
            vgemm(0, va0)
            vgemm(1, va1)
            mk(f"L{l}.attn0")
            o00 = ap.tile([P, KD, T], BF, tag="o0", name="o00", bufs=1)
            o01 = ap.tile([P, KD, T], BF, tag="o1", name="o01", bufs=1)
            attn_group(va0, va1, o00, o01)
            mk(f"L{l}.oproj0")
            oproj(0, o00, o01)
            mk(f"L{l}.lnB")
            layernorm_group(0, "u2g0")     # LN2(g0) overlaps QKV(g1)
            mk(f"L{l}.qkv1")
            qkv_rope(1)
            mk(f"L{l}.vg1")
            va0 = ap.tile([P, KD, H, HD + 1], BF, tag="va", name="va0b")
            va1 = ap.tile([P, KD, H, HD + 1], BF, tag="va", name="va1b")
            vgemm(2, va0)
            vgemm(3, va1)
            mk(f"L{l}.attn1")
            o00 = ap.tile([P, KD, T], BF, tag="o0", name="o10", bufs=1)
            o01 = ap.tile([P, KD, T], BF, tag="o1", name="o11", bufs=1)
            attn_group(va0, va1, o00, o01)
            mk(f"L{l}.oproj1")
            oproj(1, o00, o01)
            mk(f"L{l}.lnC")
            layernorm_group(1, "u2g1")     # LN2(g1) overlaps FF(g0)
            mk(f"L{l}.ff0")
            ff(0)
            if l < L - 1:
                mk(f"L{l}.lnD")
                layernorm_group(0, "nl")   # next layer LN1(g0) overlaps FF(g1)
            mk(f"L{l}.ff1")
            ff(1)

        mk("outproj")
        # ---- output projection ----
        wout_sb = singles.tile([P, KD, D_IN], BF)
        nc.sync.dma_start(wout_sb[:], w_out[:])
        for c in range(BL):
            cs = ds(c * T, T)
            acc1 = pstile("o1")
            for k in range(KD):
                nc.tensor.matmul(
                    acc1[:, 0, :], wout_sb[:, k, 0:P], h[:, k, cs],
                    start=(k == 0), stop=(k == KD - 1),
                )
            for k in range(KD):
                nc.tensor.matmul(
                    acc1[0 : D_IN - P, 1, :], wout_sb[:, k, P:D_IN],
                    h[:, k, cs],
                    start=(k == 0), stop=(k == KD - 1),
                )
            o1 = outp.tile([P, T], F32, tag="o1t", name="o1")
            nc.vector.tensor_scalar(
                o1[:], acc1[:, 0, :], bout_sb[:, 0:1], None, Alu.add
            )
            nc.sync.dma_start(out_d[0:P, cs], o1[:])
            o2 = outp.tile([P, T], F32, tag="o1t", name="o2")
            nc.vector.tensor_scalar(
                o2[0 : D_IN - P, :], acc1[0 : D_IN - P, 1, :],
                bout_sb[0 : D_IN - P, 1:2], None, Alu.add,
            )
            nc.sync.dma_start(out_d[P:D_IN, cs], o2[0 : D_IN - P, :])

    mk("end")
    _split_sync_waits(nc)
    try:
        import json

        with open("/tmp/mdm_phases.json", "w") as f:
            json.dump(_PHASES, f)
    except Exception:
        pass
    return nc


# ---------------------------------------------------------------------------
# host-side preparation
# ---------------------------------------------------------------------------

def _fm(w):
    """[K, N] -> [128, K//128, N] (partition-major k-tiles)."""
    k, n = w.shape
    return np.ascontiguousarray(w.reshape(k // P, P, n).transpose(1, 0, 2))


def _bias_fm(v):
    """[n*128] -> [128, n] feature-major per-partition columns."""
    return np.ascontiguousarray(v.reshape(-1, P).T)


def _prep_shared(inputs):
    f32 = np.float32
    g = {}

    # rope tables; sinm has the rotate-half sign pattern folded in
    inv = 1.0 / (10000.0 ** (np.arange(0, HD, 2, dtype=f32) / HD))
    ang = np.arange(T, dtype=f32)[:, None] * inv[None, :]
    ang = np.concatenate([ang, ang], axis=-1)          # [T, HD]
    cos = np.cos(ang).T.astype(f32)                    # [HD, T]
    sin = np.sin(ang).T.astype(f32)
    sgn = np.where(np.arange(HD) < HD // 2, -1.0, 1.0).astype(f32)
    sinm = sin * sgn[:, None]
    cos128 = np.concatenate([cos, cos], axis=0)        # [128, T]
    sinm128 = np.concatenate([sinm, sinm], axis=0)
    g["cos_t"] = np.repeat(cos128[:, None, :], 2, axis=1).astype(BF16)
    g["sinm_t"] = np.repeat(sinm128[:, None, :], 2, axis=1).astype(BF16)

    # timestep sinusoidal PE table for t in 0..1023
    pos = np.arange(1024, dtype=f32)[:, None]
    div = np.exp(-np.log(10000.0) * np.arange(0, D, 2, dtype=f32) / D)
    a = pos * div[None, :]
    tab = np.stack([np.sin(a), np.cos(a)], axis=-1).reshape(1024, D).astype(f32)
    g["pe_tab"] = np.ascontiguousarray(
        tab.reshape(8, P, D).transpose(1, 0, 2)
    ).astype(BF16)

    W_t1 = np.asarray(inputs["W_t1"], f32)
    W_t2 = np.asarray(inputs["W_t2"], f32)
    W_txt = np.asarray(inputs["W_txt"], f32)
    g["w_t1"] = _fm(W_t1).astype(BF16)
    g["w_t2"] = _fm(W_t2).astype(BF16)
    g["w_txt"] = _fm(W_txt / TXT).astype(BF16)
    g["bt1_fm"] = _bias_fm(np.asarray(inputs["b_t1"], f32))
    bemb = (
        np.asarray(inputs["b_t2"], f32)
        + np.asarray(inputs["b_txt"], f32)
        + np.asarray(inputs["b_in"], f32)
    )
    g["bemb_fm"] = _bias_fm(bemb)

    W_in = np.asarray(inputs["W_in"], f32)
    w_in_pad = np.zeros((2 * P, D), f32)
    w_in_pad[:D_IN] = W_in
    g["w_in"] = _fm(w_in_pad).astype(BF16)

    W_out = np.asarray(inputs["W_out"], f32)
    g["w_out"] = _fm(W_out).astype(BF16)
    b_out = np.asarray(inputs["b_out"], f32)
    bo_fm = np.zeros((P, 2), f32)
    bo_fm[:, 0] = b_out[:P]
    bo_fm[: D_IN - P, 1] = b_out[P:]
    g["bout_fm"] = bo_fm

    Wqkv = np.asarray(inputs["Wqkv"], f32)
    bqkv = np.asarray(inputs["bqkv"], f32)
    ln1_g = np.asarray(inputs["ln1_g"], f32)
    ln1_b = np.asarray(inputs["ln1_b"], f32)
    ln2_g = np.asarray(inputs["ln2_g"], f32)
    ln2_b = np.asarray(inputs["ln2_b"], f32)
    Wo = np.asarray(inputs["Wo"], f32)
    bo = np.asarray(inputs["bo"], f32)
    W1 = np.asarray(inputs["W1"], f32)
    b1 = np.asarray(inputs["b1"], f32)
    W2 = np.asarray(inputs["W2"], f32)
    b2 = np.asarray(inputs["b2"], f32)

    w_qk_l, w_v_l, w_o_l, w_1_l, w_2_l, blk_l, bv_l = [], [], [], [], [], [], []
    for l in range(L):
        Wq = Wqkv[l][:, 0:D]
        Wk = Wqkv[l][:, D : 2 * D]
        Wv = Wqkv[l][:, 2 * D : 3 * D]
        # LN gains folded into the weight rows (y on device is plain xhat)
        Wq_e = ln1_g[l][:, None] * Wq
        Wk_e = ln1_g[l][:, None] * Wk
        Wv_e = ln1_g[l][:, None] * Wv
        W1_e = ln2_g[l][:, None] * W1[l]
        bq_eff = bqkv[l][0:D] + ln1_b[l] @ Wq
        bk_eff = bqkv[l][D : 2 * D] + ln1_b[l] @ Wk
        bv_eff = bqkv[l][2 * D : 3 * D] + ln1_b[l] @ Wv
        b1_eff = b1[l] + ln2_b[l] @ W1[l]
        w_qk_l.append(_fm(np.concatenate([Wq_e, Wk_e], axis=1)).astype(BF16))
        w_v_l.append(_fm(Wv_e).astype(BF16))
        w_o_l.append(_fm(Wo[l]).astype(BF16))
        w_1_l.append(_fm(W1_e).astype(BF16))
        w_2_l.append(_fm(W2[l]).astype(BF16))
        blk_one = np.concatenate(
            [
                _bias_fm(bq_eff), _bias_fm(bk_eff),
                _bias_fm(bo[l]), _bias_fm(b1_eff), _bias_fm(b2[l]),
            ],
            axis=1,
        )  # [128, 32]
        blk_l.append(blk_one)
        bv_l.append(bv_eff)
    g["w_qk"] = np.stack(w_qk_l)
    g["w_v"] = np.stack(w_v_l)
    g["w_o"] = np.stack(w_o_l)
    g["w_1"] = np.stack(w_1_l)
    g["w_2"] = np.stack(w_2_l)
    g["blk"] = np.stack(blk_l).astype(np.float32)
    g["b_v"] = np.stack(bv_l).astype(BF16)[:, None, :]
    return g


def _prep_core(inputs, cc):
    f32 = np.float32
    d = {}
    bs = slice(cc * BL, (cc + 1) * BL)

    x = np.asarray(inputs["x"], f32)[bs]
    x_t = x.reshape(TOK, D_IN).T
    x_pad = np.zeros((2 * P, TOK), f32)
    x_pad[:D_IN] = x_t
    d["x_fm"] = x_pad.reshape(2, P, TOK).transpose(1, 0, 2).astype(BF16)

    enc = np.asarray(inputs["enc_text"], f32)[bs]
    enc_fm = enc.transpose(2, 0, 1)
    d["enc_fm"] = np.ascontiguousarray(
        enc_fm.reshape(KD, P, BL, TXT).transpose(1, 0, 2, 3)
    )

    tsv = np.asarray(inputs["timesteps"]).astype(np.int64)[bs]
    oh = np.zeros((P, 8, BL), f32)
    for j, t in enumerate(tsv):
        oh[int(t) % P, int(t) // P, j] = 1.0
    d["onehot"] = oh.astype(BF16)
    return d


_CACHE = {}


def kernel(**inputs):
    if "nc" not in _CACHE:
        _CACHE["nc"] = _build_nc()
    nc = _CACHE["nc"]

    shared = _prep_shared(inputs)
    in_maps = []
    for cc in range(NCORES):
        m = dict(shared)
        m.update(_prep_core(inputs, cc))
        in_maps.append(m)

    res = run_bass_kernel_spmd(
        nc, in_maps, core_ids=list(range(NCORES)), **_CACHE.get("run_kwargs", {})
    )
    _CACHE["last_result"] = res

    outs = []
    for cc in range(NCORES):
        o = res.results[cc]["out"]
        outs.append(o.reshape(D_IN, BL, T).transpose(1, 2, 0))
    return np.ascontiguousarray(np.concatenate(outs, axis=0), dtype=np.float32)

